# revision 13
# baseline (speedup 1.0000x reference)
"""Trainium2 Bass kernel for the CAB fusion:

    out = shallower * sigmoid(MLP(concat(gap(shallower), gap(deeper)))) +
          bilinear_upsample_2x(deeper)

Sharding: pure data parallel — batch 16 split 2-per-core across 8
NeuronCores; tiny 1x1-conv weights replicated.

Per-core dataflow (channels on partitions, 2 groups of 128):
  - deeper arrives pre-scaled by 1/16 from the host (exact power-of-2), so
    the two separable 2x-bilinear passes are pure `3*a + b`
    scalar_tensor_tensor ops on VectorE whose outputs ARE the final
    upsample values ({0.75, 0.25} = {3, 1}/4 taps per axis). Edge rows/cols
    are tiny *4 tensor_scalar ops, also on VectorE so pass-2 never waits on
    the (pool-busy) Scalar engine.
  - deeper's global-average-pool rides for free on the accum_outs of the
    four pass-2 ops: sum(upsampled) == 4*sum(deeper) exactly, and both
    mean folds collapse into a uniform w1/4096 host scale.
  - shallower is pooled on ScalarE (in-place Copy + accum_out) per loaded
    half-tile.
  - the MLP runs per-batch on TensorE, so batch 0's gate is ready mid-load
    and its finals+stores overlap batch 1's loads.
  - the fused gate+add final is one scalar_tensor_tensor per half-tile on
    VectorE (GpSimd measured 2x-slower AND mutually blocks DVE 2-read ops
    on the shared SBUF port — not used), stores pipelined per half.
  - DMAs alternate between the two HWDGE rings (sync + scalar); weight
    matrices ship as single multi-chunk DMAs to amortize the ~2us per-DMA
    fixed latency.

Numerics: fp32 end to end.
"""

import numpy as np
from contextlib import ExitStack

import concourse.bacc as bacc
import concourse.tile as tile
import concourse.mybir as mybir
from concourse import bass_utils

F32 = mybir.dt.float32
AF = mybir.ActivationFunctionType
OP = mybir.AluOpType

N_CORES = 8
B, C = 16, 256
HD, WD = 32, 32
HS, WS = 64, 64
BL = B // N_CORES          # batches per core
G = C // 128               # channel groups of 128


def _emit(ctx, tc, deeper, shallower, wpack, out):
    nc = tc.nc

    wpool = ctx.enter_context(tc.tile_pool(name="weights", bufs=1))
    stat = ctx.enter_context(tc.tile_pool(name="stat", bufs=1))
    sres = ctx.enter_context(tc.tile_pool(name="sres", bufs=1))
    dres = ctx.enter_context(tc.tile_pool(name="dres", bufs=1))
    up = ctx.enter_context(tc.tile_pool(name="up", bufs=2))
    ures = ctx.enter_context(tc.tile_pool(name="ures", bufs=4))
    psum = ctx.enter_context(tc.tile_pool(name="psum", bufs=1, space="PSUM"))

    # ---- deeper first: four 512 KB DMAs, two per HWDGE ring, so the
    # VectorE upsample starts as early as possible ----
    d_sb = []
    dview = deeper.rearrange("b c h w -> c b h w")
    for g in range(G):
        dt_ = dres.tile([128, BL, HD, WD], F32, name=f"d{g}")
        eng = nc.sync if g == 0 else nc.scalar
        for b in range(BL):
            eng.dma_start(dt_[:, b], dview[g * 128:(g + 1) * 128, b])
        d_sb.append(dt_)

    # ---- shallower: two 1 MB half-DMAs per tile, one per HWDGE ring.
    # The single packed weight DMA is interposed so it lands by the time
    # batch-0 pools do.
    s_sb = {}
    half = HS // 2

    def s_load(b, g):
        st = sres.tile([128, HS, WS], F32, name=f"s{b}{g}")
        gs = slice(g * 128, (g + 1) * 128)
        nc.sync.dma_start(st[:, 0:half, :], shallower[b, gs, 0:half, :])
        nc.scalar.dma_start(st[:, half:HS, :], shallower[b, gs, half:HS, :])
        s_sb[b, g] = st

    s_load(0, 0)

    # ---- weights: one packed DMA [128, 1540] = w1t | w2t | bias ----
    wp_sb = wpool.tile([128, 6 * C + 4], F32, name="wp_sb")
    nc.sync.dma_start(wp_sb[:], wpack[:, :])
    wmat = wp_sb[:, 0:6 * C].rearrange("p (k o) -> p k o", k=6)
    bias_sb = wp_sb[:, 6 * C:6 * C + 4]

    s_load(0, 1)

    # Preload the sigmoid LUT off the critical path (dummy op on zeros);
    # emitted after the load triggers so it doesn't delay them on ACT.
    sig_warm = stat.tile([128, 1], F32, name="sig_warm")
    nc.gpsimd.memset(sig_warm[:], 0.0)
    nc.scalar.activation(sig_warm[:], sig_warm[:], AF.Sigmoid)

    s_load(1, 0)
    s_load(1, 1)

    # ---- shallower pools: ACT in-place copy + accumulate, per half ----
    sp = [[stat.tile([128, BL], F32, name=f"sp{g}{h}") for h in range(2)]
          for g in range(G)]
    for b in range(BL):
        for g in range(G):
            st = s_sb[b, g]
            nc.scalar.activation(st[:, 0:half, :], st[:, 0:half, :], AF.Copy,
                                 accum_out=sp[g][0][:, b:b + 1])
            nc.scalar.activation(st[:, half:HS, :], st[:, half:HS, :], AF.Copy,
                                 accum_out=sp[g][1][:, b:b + 1])

    # ---- upsample: W pass then H pass, all on VectorE. Pass-2 accum_outs
    # assemble sum(upsampled) = 4*sum(deeper_scaled).
    pc = [[stat.tile([128, BL], F32, name=f"pc{g}{p}") for p in range(4)]
          for g in range(G)]
    u_sb = {}
    for b in range(BL):
        for g in range(G):
            d = d_sb[g][:, b]          # [128, 32, 32]
            yp = up.tile([128, HD, WS], F32, name="yp")
            ypv = yp.rearrange("p h (j t) -> p h j t", t=2)
            nc.vector.scalar_tensor_tensor(
                ypv[:, :, 1:WD, 0], d[:, :, 1:WD], 3.0, d[:, :, 0:WD - 1],
                OP.mult, OP.add)
            nc.vector.scalar_tensor_tensor(
                ypv[:, :, 0:WD - 1, 1], d[:, :, 0:WD - 1], 3.0, d[:, :, 1:WD],
                OP.mult, OP.add)
            nc.vector.tensor_scalar(ypv[:, :, 0, 0], d[:, :, 0], 4.0, None,
                                    OP.mult)
            nc.vector.tensor_scalar(ypv[:, :, WD - 1, 1], d[:, :, WD - 1],
                                    4.0, None, OP.mult)

            u = ures.tile([128, HS, WS], F32, name="u")
            uv = u.rearrange("p (i t) w -> p i t w", t=2)
            nc.vector.scalar_tensor_tensor(
                uv[:, 1:HD, 0, :], yp[:, 1:HD, :], 3.0, yp[:, 0:HD - 1, :],
                OP.mult, OP.add, accum_out=pc[g][0][:, b:b + 1])
            nc.vector.scalar_tensor_tensor(
                uv[:, 0:HD - 1, 1, :], yp[:, 0:HD - 1, :], 3.0, yp[:, 1:HD, :],
                OP.mult, OP.add, accum_out=pc[g][1][:, b:b + 1])
            nc.vector.tensor_scalar(uv[:, 0, 0, :], yp[:, 0, :], 4.0, 0.0,
                                    OP.mult, OP.add,
                                    accum_out=pc[g][2][:, b:b + 1])
            nc.vector.tensor_scalar(uv[:, HD - 1, 1, :], yp[:, HD - 1, :],
                                    4.0, 0.0, OP.mult, OP.add,
                                    accum_out=pc[g][3][:, b:b + 1])
            u_sb[b, g] = u

    # ---- MLP per batch, then fused gate+add finals per half + stores ----
    sig = [stat.tile([128, BL], F32, name=f"sig{g}") for g in range(G)]
    for b in range(BL):
        h_cols = []
        for og in range(G):
            ph = psum.tile([128, 1], F32, name=f"ph{og}{b}")
            ogs = slice(og * 128, (og + 1) * 128)
            chunks = []
            for g in range(G):
                for h in range(2):
                    chunks.append((wmat[:, g], sp[g][h]))
            for g in range(G):
                for p in range(4):
                    chunks.append((wmat[:, 2 + g], pc[g][p]))
            for i, (wt, col) in enumerate(chunks):
                nc.tensor.matmul(ph[:], wt[:, ogs], col[:, b:b + 1],
                                 start=(i == 0), stop=(i == len(chunks) - 1))
            ht = stat.tile([128, BL], F32, name=f"h{og}", tag=f"h{og}")
            nc.scalar.activation(ht[:, b:b + 1], ph[:], AF.Relu,
                                 bias=bias_sb[:, og:og + 1])
            h_cols.append(ht)
        for g in range(G):
            pg = psum.tile([128, 1], F32, name=f"pg{g}{b}")
            gs_ = slice(g * 128, (g + 1) * 128)
            for ig in range(G):
                nc.tensor.matmul(pg[:], wmat[:, 4 + ig, gs_],
                                 h_cols[ig][:, b:b + 1],
                                 start=(ig == 0), stop=(ig == 1))
            nc.scalar.activation(sig[g][:, b:b + 1], pg[:], AF.Sigmoid,
                                 bias=bias_sb[:, 2 + g:3 + g])

        for g in range(G):
            s = s_sb[b, g]
            u = u_sb[b, g]
            sc = sig[g][:, b:b + 1]
            gs = slice(g * 128, (g + 1) * 128)
            for h, eng in ((0, nc.sync), (1, nc.scalar)):
                rows = slice(h * half, (h + 1) * half)
                nc.vector.scalar_tensor_tensor(
                    s[:, rows, :], s[:, rows, :], sc, u[:, rows, :],
                    OP.mult, OP.add)
                eng.dma_start(out[b, gs, rows, :], s[:, rows, :])


def build_kernel():
    nc = bacc.Bacc("TRN2", target_bir_lowering=False, debug=False,
                   num_devices=N_CORES)
    deeper = nc.dram_tensor("deeper", [BL, C, HD, WD], F32,
                            kind="ExternalInput").ap()
    shallower = nc.dram_tensor("shallower", [BL, C, HS, WS], F32,
                               kind="ExternalInput").ap()
    wpack = nc.dram_tensor("wpack", [128, 6 * C + 4], F32,
                           kind="ExternalInput").ap()
    out = nc.dram_tensor("out", [BL, C, HS, WS], F32,
                         kind="ExternalOutput").ap()

    with tile.TileContext(nc) as tc, ExitStack() as ctx:
        _emit(ctx, tc, deeper, shallower, wpack, out)
    nc.compile()
    return nc


_NC = None


def _get_nc():
    global _NC
    if _NC is None:
        _NC = build_kernel()
    return _NC


def prepare_in_maps(deeper, shallower, w1, b1, w2, b2):
    # w1t: transposed, uniform /4096 (shallow 1/(64*64) mean fold; deeper
    # 1/(32*32)/4 sum(U)-to-sum(X) fold — both equal 1/4096).
    w1t = (np.ascontiguousarray(w1.T).astype(np.float32)
           * np.float32(1.0 / 4096.0))                    # [512, 256]
    w2t = np.ascontiguousarray(w2.T).astype(np.float32)   # [256, 256]
    # wpack[p, :] = w1t 4 chunks | w2t 2 chunks | bias cols
    wpack = np.empty((128, 6 * C + 4), np.float32)
    for k in range(4):
        wpack[:, k * C:(k + 1) * C] = w1t[k * 128:(k + 1) * 128]
    for k in range(2):
        wpack[:, (4 + k) * C:(5 + k) * C] = w2t[k * 128:(k + 1) * 128]
    b1f = b1.astype(np.float32).reshape(2, 128)
    b2f = b2.astype(np.float32).reshape(2, 128)
    wpack[:, 6 * C + 0] = b1f[0]
    wpack[:, 6 * C + 1] = b1f[1]
    wpack[:, 6 * C + 2] = b2f[0]
    wpack[:, 6 * C + 3] = b2f[1]
    d16 = (deeper.astype(np.float32) * np.float32(1.0 / 16.0))
    in_maps = []
    for i in range(N_CORES):
        in_maps.append({
            "deeper": np.ascontiguousarray(d16[i * BL:(i + 1) * BL]),
            "shallower": np.ascontiguousarray(shallower[i * BL:(i + 1) * BL]),
            "wpack": wpack,
        })
    return in_maps


def gather(results):
    return np.concatenate([results[i]["out"] for i in range(N_CORES)], axis=0)


def kernel(deeper, shallower, w1, b1, w2, b2):
    nc = _get_nc()
    in_maps = prepare_in_maps(deeper, shallower, w1, b1, w2, b2)
    res = bass_utils.run_bass_kernel_spmd(nc, in_maps, list(range(N_CORES)))
    return gather(res.results)
